# revision 11
# baseline (speedup 1.0000x reference)
"""Luong concat attention with ragged per-tree segments, on 8 TRN2 NeuronCores.

Math (reference):
    rep    = prev_hidden_states[segment_ids]               # [N, H]
    energy = tanh(rep @ W1.T + enc @ W2.T + b)             # [N, H]
    scores = (energy @ v)[:, 0]                            # [N]
    attn   = segmented_softmax(scores, segment_ids)        # [N, 1]

Distribution: segments are contiguous runs of nodes (segment_ids sorted), so we
shard whole segments across the 8 cores (balanced contiguous ranges, padded to
a common length P).  No cross-core collective is needed: every segment lives on
exactly one core.

Per-core device kernel (SPMD, one program):
  - ph1 = prev @ W1.T + b computed on-device, laid out [seg=64 part, H free].
  - energy^T tiles [H part(4x128), nodes 512 free] via f32r matmuls:
    K-chunks of W2^T against enc^T tiles, plus a K=64 "one-hot" matmul that
    adds ph1[seg[n]] without a gather.
  - scores broadcast to 64 partitions by using v replicated 64x as lhsT; a
    one-hot-derived {0,-BIG} mask is added so row s holds scores only where
    segment==s; per-segment max/sum then become plain free-dim reductions.
  - segmented softmax: masked-max -> exp(x - m) with per-partition bias
    (ACT accum_out gives the per-tile sums for free) -> colsum matmul with
    lhsT = 1/denom folds normalization and the 64->1 partition reduction.
Pad columns have all-zero one-hot -> masked to -BIG -> contribute nothing.
"""

import sys

sys.path.insert(0, "/opt/trn_rl_repo")

import numpy as np

import concourse.bass as bass
import concourse.tile as tile
from concourse import bacc, mybir
from concourse.bass import ts
from concourse.bass_utils import run_bass_kernel_spmd

B = 64
N_TOTAL = 65536
H = 512
NCORES = 8
TILE_N = 512
F32 = mybir.dt.float32
F32R = mybir.dt.float32r
BIG = float(2.0**30)

LAST_RESULTS = None  # BassKernelResults of the most recent run (for test harness)
_NC_CACHE: dict = {}


def build_nc(P: int):
    """Build + compile the SPMD program for per-core padded node count P."""
    import os
    STAGE = int(os.environ.get("K_STAGE", "4"))
    SUB = int(os.environ.get("K_SUB", "9"))
    NT = P // TILE_N
    nc = bacc.Bacc("TRN2", target_bir_lowering=False, debug=False)

    encT_d = nc.dram_tensor("encT", [H, P], F32R, kind="ExternalInput")
    oh_d = nc.dram_tensor("oh", [B, P], F32R, kind="ExternalInput")
    w1t_d = nc.dram_tensor("w1t", [H, H], F32R, kind="ExternalInput")
    w2t_d = nc.dram_tensor("w2t", [H, H], F32R, kind="ExternalInput")
    prevT_d = nc.dram_tensor("prevT", [H, B], F32R, kind="ExternalInput")
    vrep_d = nc.dram_tensor("vrep", [H, B], F32R, kind="ExternalInput")
    b_d = nc.dram_tensor("b", [1, H], F32R, kind="ExternalInput")
    ones_d = nc.dram_tensor("ones", [1, B], F32R, kind="ExternalInput")
    attn_d = nc.dram_tensor("attn", [1, P], F32, kind="ExternalOutput")

    with tile.TileContext(nc) as tc:
        with (
            nc.allow_low_precision(reason="f32r tiles are 4-byte fp32 storage"),
            tc.tile_pool(name="const", bufs=1) as const,
            tc.tile_pool(name="keep", bufs=1) as keep,
            tc.tile_pool(name="enc", bufs=3) as enc_pool,
            tc.tile_pool(name="oh", bufs=3) as oh_pool,
            tc.tile_pool(name="tanh", bufs=2) as tanh_pool,
            tc.tile_pool(name="tmp", bufs=3) as tmp_pool,
            tc.tile_pool(name="ps_e", bufs=4, space="PSUM") as ps_e,
            tc.tile_pool(name="ps_s", bufs=2, space="PSUM") as ps_s,
            tc.tile_pool(name="ps_a", bufs=2, space="PSUM") as ps_a,
        ):
            # ---- constants / small tensors ----
            w1t_sb = const.tile([128, 4, H], F32R)
            nc.sync.dma_start(out=w1t_sb, in_=w1t_d[:].rearrange("(kc p) j -> p kc j", p=128))
            w2t_sb = const.tile([128, 4, H], F32R)
            nc.sync.dma_start(out=w2t_sb, in_=w2t_d[:].rearrange("(kc p) j -> p kc j", p=128))
            prevT_sb = const.tile([128, 4, B], F32R)
            nc.sync.dma_start(out=prevT_sb, in_=prevT_d[:].rearrange("(kc p) j -> p kc j", p=128))
            vrep_sb = const.tile([128, 4, B], F32R)
            nc.sync.dma_start(out=vrep_sb, in_=vrep_d[:].rearrange("(kc p) j -> p kc j", p=128))
            b_sb = const.tile([1, H], F32R)
            nc.sync.dma_start(out=b_sb, in_=b_d[:])
            ones_sb = const.tile([1, B], F32R)
            nc.sync.dma_start(out=ones_sb, in_=ones_d[:])

            # ---- ph1 = prev @ W1.T + b, laid out [seg, h_out] ----
            ph1_ps = ps_s.tile([B, H], F32, tag="s")
            for kc in range(4):
                nc.tensor.matmul(
                    ph1_ps, lhsT=(prevT_sb[:, kc, :]), rhs=(w1t_sb[:, kc, :]),
                    start=(kc == 0), stop=False,
                )
            nc.tensor.matmul(ph1_ps, lhsT=(ones_sb), rhs=(b_sb), start=False, stop=True)
            ph1_sb = const.tile([B, H], F32R)
            nc.vector.tensor_copy(ph1_sb, ph1_ps)

            # ---- persistent accumulators ----
            masked_all = keep.tile([B, NT, TILE_N], F32)
            e_all = keep.tile([B, NT, TILE_N], F32R)
            mparts = keep.tile([B, NT], F32)
            ssum = keep.tile([B, NT], F32)
            m_acc = keep.tile([B, 1], F32)
            negm = keep.tile([B, 1], F32)
            denom = keep.tile([B, 1], F32)
            dinv = keep.tile([B, 1], F32R)
            out_sb = keep.tile([1, P], F32)

            encT_v = encT_d[:].rearrange("(kc p) n -> p kc n", p=128)

            # ---- pass 1: scores + masked + per-tile max ----
            for t in range(NT):
                enc_sb = enc_pool.tile([128, 4, TILE_N], F32R)
                nc.sync.dma_start(out=enc_sb, in_=encT_v[:, :, ts(t, TILE_N)])
                oh_sb = oh_pool.tile([B, TILE_N], F32R)
                nc.sync.dma_start(out=oh_sb, in_=oh_d[:, ts(t, TILE_N)])

                tanh_sb = tanh_pool.tile([128, 4, TILE_N], F32R)
                for hc in range(4):
                    eps = ps_e.tile([128, TILE_N], F32)
                    for kc in range(4):
                        nc.tensor.matmul(
                            eps,
                            lhsT=(w2t_sb[:, kc, ts(hc, 128)]),
                            rhs=(enc_sb[:, kc, :]),
                            start=(kc == 0), stop=False,
                        )
                    if SUB >= 2:
                        nc.tensor.matmul(
                            eps, lhsT=(ph1_sb[:, ts(hc, 128)]), rhs=(oh_sb),
                            start=False, stop=True,
                        )
                    else:
                        nc.tensor.matmul(
                            eps, lhsT=(w2t_sb[:, 0, ts(hc, 128)]), rhs=(enc_sb[:, 0, :]),
                            start=False, stop=True,
                        )
                    nc.scalar.activation(
                        out=tanh_sb[:, hc, :], in_=eps,
                        func=mybir.ActivationFunctionType.Tanh,
                    )

                if SUB < 3:
                    continue
                spsum = ps_s.tile([B, TILE_N], F32, tag="s")
                for kc in range(4):
                    nc.tensor.matmul(
                        spsum, lhsT=(vrep_sb[:, kc, :]), rhs=(tanh_sb[:, kc, :]),
                        start=(kc == 0), stop=(kc == 3),
                    )

                if SUB < 4:
                    continue
                # ohm = oh*BIG - BIG  (0 where member, -BIG where not)
                ohm_sb = tmp_pool.tile([B, TILE_N], F32)
                nc.vector.tensor_scalar(
                    out=ohm_sb, in0=oh_sb, scalar1=BIG, scalar2=BIG,
                    op0=mybir.AluOpType.mult, op1=mybir.AluOpType.subtract,
                )
                # masked = scores + ohm ; mparts[:, t] = max(masked)
                nc.vector.tensor_tensor(
                    out=masked_all[:, t, :], in0=spsum, in1=ohm_sb,
                    op=mybir.AluOpType.add,
                )
                nc.vector.reduce_max(
                    out=mparts[:, t : t + 1], in_=masked_all[:, t, :],
                    axis=mybir.AxisListType.X,
                )

            # ---- segment max across tiles; bias = min(-m, 1e6) ----
            if STAGE >= 2:
                nc.vector.reduce_max(out=m_acc, in_=mparts, axis=mybir.AxisListType.X)
                nc.vector.tensor_scalar(
                    out=negm, in0=m_acc, scalar1=-1.0, scalar2=1e6,
                    op0=mybir.AluOpType.mult, op1=mybir.AluOpType.min,
                )

            # ---- pass 2: e = exp(masked - m); per-tile sums via accum_out ----
            if STAGE >= 3:
                for t in range(NT):
                    nc.scalar.activation(
                        out=e_all[:, t, :], in_=masked_all[:, t, :],
                        func=mybir.ActivationFunctionType.Exp,
                        bias=negm, scale=1.0,
                        accum_out=ssum[:, t : t + 1],
                    )

                nc.vector.reduce_sum(out=denom, in_=ssum, axis=mybir.AxisListType.X)
                nc.vector.tensor_scalar_add(out=denom, in0=denom, scalar1=1e-30)
                nc.vector.reciprocal(out=dinv, in_=denom)

            # ---- pass 3: attn = colsum(dinv[s] * e[s, n]) ----
            if STAGE >= 4:
                for t in range(NT):
                    apsum = ps_a.tile([1, TILE_N], F32)
                    nc.tensor.matmul(
                        apsum, lhsT=(dinv), rhs=(e_all[:, t, :]), start=True, stop=True
                    )
                    nc.scalar.activation(
                        out=out_sb[:, ts(t, TILE_N)], in_=apsum,
                        func=mybir.ActivationFunctionType.Copy,
                    )
            else:
                nc.vector.memset(out_sb, 0.0)

            nc.sync.dma_start(out=attn_d[:], in_=out_sb)

    nc.compile()
    return nc


def _plan_shards(seg: np.ndarray):
    """Contiguous, segment-aligned split of nodes into NCORES groups."""
    counts = np.bincount(seg, minlength=B).astype(np.int64)
    cum = np.concatenate([[0], np.cumsum(counts)])  # [B+1]
    n = int(cum[-1])
    bounds = [0]
    for c in range(1, NCORES):
        ideal = n * c / NCORES
        s = int(np.argmin(np.abs(cum - ideal)))
        s = max(s, bounds[-1] + 1) if B - s >= NCORES - c else s
        s = min(max(s, bounds[-1]), B - (NCORES - c))
        if s <= bounds[-1]:
            s = bounds[-1] + 1
        bounds.append(s)
    bounds.append(B)
    starts = [int(cum[bounds[c]]) for c in range(NCORES)]
    lens = [int(cum[bounds[c + 1]] - cum[bounds[c]]) for c in range(NCORES)]
    return starts, lens


def kernel(prev_hidden_states, encoder_output, segment_ids, W, b, v):
    global LAST_RESULTS
    prev = np.ascontiguousarray(np.asarray(prev_hidden_states, dtype=np.float32))
    enc = np.ascontiguousarray(np.asarray(encoder_output, dtype=np.float32))
    seg = np.asarray(segment_ids)
    seg_i = seg.astype(np.int64)
    W_np = np.asarray(W, dtype=np.float32)
    b_np = np.asarray(b, dtype=np.float32)
    v_np = np.asarray(v, dtype=np.float32)
    n_total = enc.shape[0]

    starts, lens = _plan_shards(seg_i)
    P = int(np.ceil(max(lens) / TILE_N) * TILE_N)
    P = max(P, TILE_N)

    if P not in _NC_CACHE:
        _NC_CACHE[P] = build_nc(P)
    nc = _NC_CACHE[P]

    encT = np.ascontiguousarray(enc.T)  # [H, N]
    w1t = np.ascontiguousarray(W_np[:, :H].T)
    w2t = np.ascontiguousarray(W_np[:, H:].T)
    prevT = np.ascontiguousarray(prev.T)
    vrep = np.ascontiguousarray(np.repeat(v_np.reshape(H, 1), B, axis=1))
    b_row = np.ascontiguousarray(b_np.reshape(1, H))

    in_maps = []
    for c in range(NCORES):
        o, L = starts[c], lens[c]
        encT_c = np.zeros((H, P), dtype=np.float32)
        encT_c[:, :L] = encT[:, o : o + L]
        oh_c = np.zeros((B, P), dtype=np.float32)
        if L > 0:
            oh_c[seg_i[o : o + L], np.arange(L)] = 1.0
        in_maps.append(
            {
                "encT": encT_c,
                "oh": oh_c,
                "w1t": w1t,
                "w2t": w2t,
                "prevT": prevT,
                "vrep": vrep,
                "b": b_row,
                "ones": np.ones((1, B), dtype=np.float32),
            }
        )

    import os

    res = run_bass_kernel_spmd(
        nc, in_maps, core_ids=list(range(NCORES)),
        trace=bool(os.environ.get("BASS_TRACE")),
    )
    LAST_RESULTS = res

    out = np.zeros((n_total, 1), dtype=np.float32)
    for c in range(NCORES):
        o, L = starts[c], lens[c]
        if L > 0:
            out[o : o + L, 0] = res.results[c]["attn"][0, :L]
    return out


# revision 12
# speedup vs baseline: 1.0072x; 1.0072x over previous
"""Luong concat attention with ragged per-tree segments, on 8 TRN2 NeuronCores.

Math (reference):
    rep    = prev_hidden_states[segment_ids]               # [N, H]
    energy = tanh(rep @ W1.T + enc @ W2.T + b)             # [N, H]
    scores = (energy @ v)[:, 0]                            # [N]
    attn   = segmented_softmax(scores, segment_ids)        # [N, 1]

Distribution: segments are contiguous runs of nodes (segment_ids sorted), so we
shard whole segments across the 8 cores (balanced contiguous ranges, padded to
a common length P).  No cross-core collective is needed: every segment lives on
exactly one core.

Per-core device kernel (SPMD, one program):
  - ph1 = prev @ W1.T + b computed on-device, laid out [seg=64 part, H free].
  - energy^T tiles [H part(4x128), nodes 512 free] via f32r matmuls:
    K-chunks of W2^T against enc^T tiles, plus a K=64 "one-hot" matmul that
    adds ph1[seg[n]] without a gather.
  - scores broadcast to 64 partitions by using v replicated 64x as lhsT; a
    one-hot-derived {0,-BIG} mask is added so row s holds scores only where
    segment==s; per-segment max/sum then become plain free-dim reductions.
  - segmented softmax: masked-max -> exp(x - m) with per-partition bias
    (ACT accum_out gives the per-tile sums for free) -> colsum matmul with
    lhsT = 1/denom folds normalization and the 64->1 partition reduction.
Pad columns have all-zero one-hot -> masked to -BIG -> contribute nothing.
"""

import sys

sys.path.insert(0, "/opt/trn_rl_repo")

import numpy as np

import concourse.bass as bass
import concourse.tile as tile
from concourse import bacc, mybir
from concourse.bass import ts
from concourse.bass_utils import run_bass_kernel_spmd

B = 64
N_TOTAL = 65536
H = 512
NCORES = 8
TILE_N = 512
F32 = mybir.dt.float32
F32R = mybir.dt.float32r
BIG = float(2.0**30)

LAST_RESULTS = None  # BassKernelResults of the most recent run (for test harness)
_NC_CACHE: dict = {}


def build_nc(P: int):
    """Build + compile the SPMD program for per-core padded node count P."""
    import os
    STAGE = int(os.environ.get("K_STAGE", "4"))
    SUB = int(os.environ.get("K_SUB", "9"))
    NT = P // TILE_N
    nc = bacc.Bacc("TRN2", target_bir_lowering=False, debug=False)

    encT_d = nc.dram_tensor("encT", [H, P], F32R, kind="ExternalInput")
    oh_d = nc.dram_tensor("oh", [B, P], F32R, kind="ExternalInput")
    w1t_d = nc.dram_tensor("w1t", [H, H], F32R, kind="ExternalInput")
    w2t_d = nc.dram_tensor("w2t", [H, H], F32R, kind="ExternalInput")
    prevT_d = nc.dram_tensor("prevT", [H, B], F32R, kind="ExternalInput")
    vrep_d = nc.dram_tensor("vrep", [H, B], F32R, kind="ExternalInput")
    b_d = nc.dram_tensor("b", [1, H], F32R, kind="ExternalInput")
    ones_d = nc.dram_tensor("ones", [1, B], F32R, kind="ExternalInput")
    attn_d = nc.dram_tensor("attn", [1, P], F32, kind="ExternalOutput")

    with tile.TileContext(nc) as tc:
        with (
            nc.allow_low_precision(reason="f32r tiles are 4-byte fp32 storage"),
            tc.tile_pool(name="const", bufs=1) as const,
            tc.tile_pool(name="keep", bufs=1) as keep,
            tc.tile_pool(name="enc", bufs=4) as enc_pool,
            tc.tile_pool(name="oh", bufs=4) as oh_pool,
            tc.tile_pool(name="tanh", bufs=3) as tanh_pool,
            tc.tile_pool(name="tmp", bufs=3) as tmp_pool,
            tc.tile_pool(name="ps_e", bufs=4, space="PSUM") as ps_e,
            tc.tile_pool(name="ps_s", bufs=2, space="PSUM") as ps_s,
            tc.tile_pool(name="ps_a", bufs=2, space="PSUM") as ps_a,
        ):
            # ---- constants / small tensors ----
            w1t_sb = const.tile([128, 4, H], F32R)
            nc.sync.dma_start(out=w1t_sb, in_=w1t_d[:].rearrange("(kc p) j -> p kc j", p=128))
            w2t_sb = const.tile([128, 4, H], F32R)
            nc.sync.dma_start(out=w2t_sb, in_=w2t_d[:].rearrange("(kc p) j -> p kc j", p=128))
            prevT_sb = const.tile([128, 4, B], F32R)
            nc.sync.dma_start(out=prevT_sb, in_=prevT_d[:].rearrange("(kc p) j -> p kc j", p=128))
            vrep_sb = const.tile([128, 4, B], F32R)
            nc.sync.dma_start(out=vrep_sb, in_=vrep_d[:].rearrange("(kc p) j -> p kc j", p=128))
            b_sb = const.tile([1, H], F32R)
            nc.sync.dma_start(out=b_sb, in_=b_d[:])
            ones_sb = const.tile([1, B], F32R)
            nc.sync.dma_start(out=ones_sb, in_=ones_d[:])

            # ---- ph1 = prev @ W1.T + b, laid out [seg, h_out] ----
            ph1_ps = ps_s.tile([B, H], F32, tag="s")
            for kc in range(4):
                nc.tensor.matmul(
                    ph1_ps, lhsT=(prevT_sb[:, kc, :]), rhs=(w1t_sb[:, kc, :]),
                    start=(kc == 0), stop=False,
                )
            nc.tensor.matmul(ph1_ps, lhsT=(ones_sb), rhs=(b_sb), start=False, stop=True)
            ph1_sb = const.tile([B, H], F32R)
            nc.vector.tensor_copy(ph1_sb, ph1_ps)

            # ---- persistent accumulators ----
            masked_all = keep.tile([B, NT, TILE_N], F32)
            e_all = keep.tile([B, NT, TILE_N], F32R)
            mparts = keep.tile([B, NT], F32)
            ssum = keep.tile([B, NT], F32)
            m_acc = keep.tile([B, 1], F32)
            negm = keep.tile([B, 1], F32)
            denom = keep.tile([B, 1], F32)
            dinv = keep.tile([B, 1], F32R)
            out_sb = keep.tile([1, P], F32)

            encT_v = encT_d[:].rearrange("(kc p) n -> p kc n", p=128)

            # ---- pass 1: scores + masked + per-tile max ----
            for t in range(NT):
                enc_sb = enc_pool.tile([128, 4, TILE_N], F32R)
                nc.sync.dma_start(out=enc_sb, in_=encT_v[:, :, ts(t, TILE_N)])
                oh_sb = oh_pool.tile([B, TILE_N], F32R)
                nc.sync.dma_start(out=oh_sb, in_=oh_d[:, ts(t, TILE_N)])

                tanh_sb = tanh_pool.tile([128, 4, TILE_N], F32R)
                for hc in range(4):
                    eps = ps_e.tile([128, TILE_N], F32)
                    for kc in range(4):
                        nc.tensor.matmul(
                            eps,
                            lhsT=(w2t_sb[:, kc, ts(hc, 128)]),
                            rhs=(enc_sb[:, kc, :]),
                            start=(kc == 0), stop=False,
                        )
                    if SUB >= 2:
                        nc.tensor.matmul(
                            eps, lhsT=(ph1_sb[:, ts(hc, 128)]), rhs=(oh_sb),
                            start=False, stop=True,
                        )
                    else:
                        nc.tensor.matmul(
                            eps, lhsT=(w2t_sb[:, 0, ts(hc, 128)]), rhs=(enc_sb[:, 0, :]),
                            start=False, stop=True,
                        )
                    nc.scalar.activation(
                        out=tanh_sb[:, hc, :], in_=eps,
                        func=mybir.ActivationFunctionType.Tanh,
                    )

                if SUB < 3:
                    continue
                spsum = ps_s.tile([B, TILE_N], F32, tag="s")
                for kc in range(4):
                    nc.tensor.matmul(
                        spsum, lhsT=(vrep_sb[:, kc, :]), rhs=(tanh_sb[:, kc, :]),
                        start=(kc == 0), stop=(kc == 3),
                    )

                if SUB < 4:
                    continue
                # ohm = oh*BIG - BIG  (0 where member, -BIG where not)
                ohm_sb = tmp_pool.tile([B, TILE_N], F32)
                nc.vector.tensor_scalar(
                    out=ohm_sb, in0=oh_sb, scalar1=BIG, scalar2=BIG,
                    op0=mybir.AluOpType.mult, op1=mybir.AluOpType.subtract,
                )
                # masked = scores + ohm ; mparts[:, t] = max(masked)
                nc.vector.tensor_tensor(
                    out=masked_all[:, t, :], in0=spsum, in1=ohm_sb,
                    op=mybir.AluOpType.add,
                )
                nc.vector.reduce_max(
                    out=mparts[:, t : t + 1], in_=masked_all[:, t, :],
                    axis=mybir.AxisListType.X,
                )

            # ---- segment max across tiles; bias = min(-m, 1e6) ----
            if STAGE >= 2:
                nc.vector.reduce_max(out=m_acc, in_=mparts, axis=mybir.AxisListType.X)
                nc.vector.tensor_scalar(
                    out=negm, in0=m_acc, scalar1=-1.0, scalar2=1e6,
                    op0=mybir.AluOpType.mult, op1=mybir.AluOpType.min,
                )

            # ---- pass 2: e = exp(masked - m); per-tile sums via accum_out ----
            if STAGE >= 3:
                for t in range(NT):
                    nc.scalar.activation(
                        out=e_all[:, t, :], in_=masked_all[:, t, :],
                        func=mybir.ActivationFunctionType.Exp,
                        bias=negm, scale=1.0,
                        accum_out=ssum[:, t : t + 1],
                    )

                nc.vector.reduce_sum(out=denom, in_=ssum, axis=mybir.AxisListType.X)
                nc.vector.tensor_scalar_add(out=denom, in0=denom, scalar1=1e-30)
                nc.vector.reciprocal(out=dinv, in_=denom)

            # ---- pass 3: attn = colsum(dinv[s] * e[s, n]) ----
            if STAGE >= 4:
                for t in range(NT):
                    apsum = ps_a.tile([1, TILE_N], F32)
                    nc.tensor.matmul(
                        apsum, lhsT=(dinv), rhs=(e_all[:, t, :]), start=True, stop=True
                    )
                    nc.vector.tensor_copy(out_sb[:, ts(t, TILE_N)], apsum)
            else:
                nc.vector.memset(out_sb, 0.0)

            nc.sync.dma_start(out=attn_d[:], in_=out_sb)

    nc.compile()
    return nc


def _plan_shards(seg: np.ndarray):
    """Contiguous, segment-aligned split of nodes into NCORES groups."""
    counts = np.bincount(seg, minlength=B).astype(np.int64)
    cum = np.concatenate([[0], np.cumsum(counts)])  # [B+1]
    n = int(cum[-1])
    bounds = [0]
    for c in range(1, NCORES):
        ideal = n * c / NCORES
        s = int(np.argmin(np.abs(cum - ideal)))
        s = max(s, bounds[-1] + 1) if B - s >= NCORES - c else s
        s = min(max(s, bounds[-1]), B - (NCORES - c))
        if s <= bounds[-1]:
            s = bounds[-1] + 1
        bounds.append(s)
    bounds.append(B)
    starts = [int(cum[bounds[c]]) for c in range(NCORES)]
    lens = [int(cum[bounds[c + 1]] - cum[bounds[c]]) for c in range(NCORES)]
    return starts, lens


def kernel(prev_hidden_states, encoder_output, segment_ids, W, b, v):
    global LAST_RESULTS
    prev = np.ascontiguousarray(np.asarray(prev_hidden_states, dtype=np.float32))
    enc = np.ascontiguousarray(np.asarray(encoder_output, dtype=np.float32))
    seg = np.asarray(segment_ids)
    seg_i = seg.astype(np.int64)
    W_np = np.asarray(W, dtype=np.float32)
    b_np = np.asarray(b, dtype=np.float32)
    v_np = np.asarray(v, dtype=np.float32)
    n_total = enc.shape[0]

    starts, lens = _plan_shards(seg_i)
    P = int(np.ceil(max(lens) / TILE_N) * TILE_N)
    P = max(P, TILE_N)

    if P not in _NC_CACHE:
        _NC_CACHE[P] = build_nc(P)
    nc = _NC_CACHE[P]

    encT = np.ascontiguousarray(enc.T)  # [H, N]
    w1t = np.ascontiguousarray(W_np[:, :H].T)
    w2t = np.ascontiguousarray(W_np[:, H:].T)
    prevT = np.ascontiguousarray(prev.T)
    vrep = np.ascontiguousarray(np.repeat(v_np.reshape(H, 1), B, axis=1))
    b_row = np.ascontiguousarray(b_np.reshape(1, H))

    in_maps = []
    for c in range(NCORES):
        o, L = starts[c], lens[c]
        encT_c = np.zeros((H, P), dtype=np.float32)
        encT_c[:, :L] = encT[:, o : o + L]
        oh_c = np.zeros((B, P), dtype=np.float32)
        if L > 0:
            oh_c[seg_i[o : o + L], np.arange(L)] = 1.0
        in_maps.append(
            {
                "encT": encT_c,
                "oh": oh_c,
                "w1t": w1t,
                "w2t": w2t,
                "prevT": prevT,
                "vrep": vrep,
                "b": b_row,
                "ones": np.ones((1, B), dtype=np.float32),
            }
        )

    import os

    res = run_bass_kernel_spmd(
        nc, in_maps, core_ids=list(range(NCORES)),
        trace=bool(os.environ.get("BASS_TRACE")),
    )
    LAST_RESULTS = res

    out = np.zeros((n_total, 1), dtype=np.float32)
    for c in range(NCORES):
        o, L = starts[c], lens[c]
        if L > 0:
            out[o : o + L, 0] = res.results[c]["attn"][0, :L]
    return out
